# revision 1
# baseline (speedup 1.0000x reference)
"""Trainium2 Bass kernel for nn_DiscreteWaveletTransform (3-level db4 DWT,
symmetric padding, + linear resize of each coefficient band back to T).

Approach: the whole per-signal pipeline (3 DWT levels + 4 resizes) is one
fixed linear operator out[t, c] = sum_k sig[k] * M[k, 4t+c].  M (2048 x 8192)
is banded (bandwidth <= 194 rows per 128-wide t-chunk), so each 128-signal
block reduces to 46 PE matmuls of [K=128, M=128 signals, N=512 cols] in
float32r (full-rate fp32 with 11-bit mantissa), accumulated in PSUM over the
2-3 k-blocks that cover each t-chunk's band.  The matmul orientation puts
signals on PSUM partitions and (t, c)-interleaved columns on the free axis,
so each PSUM tile is exactly a contiguous [128 signals, 128 t x 4 c] chunk of
the output layout — no transposes and fully contiguous 2 KiB-per-row output
DMAs.

Sharding: data-parallel over B (16 -> 2 per core); each core handles
2 x 512 = 1024 signals = 8 blocks of 128.
"""

from contextlib import ExitStack

import numpy as np

import concourse.bacc as bacc
import concourse.bass as bass
import concourse.tile as tile
from concourse import mybir
from concourse.bass_utils import run_bass_kernel_spmd

# ---------------------------------------------------------------- problem dims
import os  # noqa: E402

B, T, N = 16, 2048, 512
LEVELS = 3
C = LEVELS + 1
F = 8
NCORES = 8
B_PER_CORE = B // NCORES          # 2
TCHUNKS = T // 128                # 16
SBLOCKS = B_PER_CORE * (N // 128)  # 8 signal blocks of 128 per core
OGROUP = int(os.environ.get("K_OGROUP", "4"))  # t-chunks per output DMA

DEC_LO = np.array([-0.010597401784997278, 0.032883011666982945, 0.030841381835986965,
                   -0.18703481171888114, -0.02798376941698385, 0.6308807679295904,
                   0.7148465705525415, 0.23037781330885523])
DEC_HI = np.array([-0.23037781330885523, 0.7148465705525415, -0.6308807679295904,
                   -0.02798376941698385, 0.18703481171888114, 0.030841381835986965,
                   -0.032883011666982945, -0.010597401784997278])


# ------------------------------------------------------- operator construction
def _dwt_step(sig, lo, hi):
    S = sig.shape[1]
    ext = np.pad(sig, ((0, 0), (F - 1, F - 1)), mode='symmetric')[:, 1:]
    L = (S + F - 1) // 2
    lo_r, hi_r = lo[::-1], hi[::-1]
    cA = sum(ext[:, k:k + 2 * L:2] * lo_r[k] for k in range(F))
    cD = sum(ext[:, k:k + 2 * L:2] * hi_r[k] for k in range(F))
    return cA, cD


def _resize(c, t):
    S = c.shape[-1]
    if S == t:
        return c
    if S > t:
        return c[..., :t]
    pos = (np.arange(t, dtype=c.dtype) + 0.5) * (S / t) - 0.5
    pos = np.clip(pos, 0.0, S - 1)
    lo = np.floor(pos).astype(np.int64)
    hi = np.minimum(lo + 1, S - 1)
    w = pos - lo.astype(c.dtype)
    return c[..., lo] * (1.0 - w) + c[..., hi] * w


def _build_operator():
    """M [T, T, C] float64: out[s, t, c] = sum_k sig[s, k] M[k, t, c]."""
    a = np.eye(T)
    details = []
    for _ in range(LEVELS):
        a, d = _dwt_step(a, DEC_LO, DEC_HI)
        details.append(d)
    coeffs = [a] + details[::-1]
    return np.stack([_resize(cf, T) for cf in coeffs], axis=-1)


def _plan():
    """Banded matmul schedule: per t-chunk, the 128-aligned k-blocks whose
    rows are nonzero, and the packed-M array [128, n_pairs, 512] f32."""
    M = _build_operator()                      # [k, t, c]
    Mi = M.reshape(T, T * C)                   # col = 4 t + c
    kblocks, pairs = [], []
    for tc in range(TCHUNKS):
        cols = Mi[:, tc * 512:(tc + 1) * 512]
        rows = np.nonzero(np.any(cols != 0, axis=1))[0]
        blocks = list(range(rows.min() // 128, rows.max() // 128 + 1))
        kblocks.append(blocks)
        for kb in blocks:
            pairs.append((tc, kb))
    packed = np.empty((128, len(pairs), 512), dtype=np.float32)
    for p, (tc, kb) in enumerate(pairs):
        packed[:, p, :] = Mi[kb * 128:(kb + 1) * 128, tc * 512:(tc + 1) * 512]
    return kblocks, pairs, packed


_KBLOCKS, _PAIRS, _M_PACKED = _plan()
NPAIRS = len(_PAIRS)

F32 = mybir.dt.float32
F32R = mybir.dt.float32r
BF16 = mybir.dt.bfloat16

# M in bf16 halves the dominant input stream (11.75 -> 5.9 MiB per core).
# Walrus rejects mixed-dtype matmuls, so the signal is cast to bf16 in the
# load DMA (SWDGE casting DMA) when M is bf16.  float32r/float32r is the
# high-accuracy fallback (2.1e-4 vs ~1e-3 relative error).
_DT_CHOICE = os.environ.get("K_DTYPE", "bf16")
M_DTYPE = BF16 if _DT_CHOICE == "bf16" else F32R
SIG_DTYPE = M_DTYPE

if M_DTYPE == BF16:
    import ml_dtypes
    _M_PACKED = _M_PACKED.astype(ml_dtypes.bfloat16)


# ------------------------------------------------------------- device program
def _emit_body(tc_ctx, nc, x_d, m_d, o_d, pools, ogroup=None):
    mpool, spool, opool, ppool = pools

    m_t = mpool.tile([128, NPAIRS, 512], M_DTYPE, name="m_t")
    pair_idx = {pr: i for i, pr in enumerate(_PAIRS)}
    m_loaded = [False] * NPAIRS
    ncopy = 0
    for b in range(B_PER_CORE):
        for nb in range(N // 128):
            sig = spool.tile([128, TCHUNKS, 128], SIG_DTYPE, name="sig")
            nc.sync.dma_start(sig[:], x_d[b, nb])
            for tci in range(TCHUNKS):
                blocks = _KBLOCKS[tci]
                # just-in-time M loads: emitted right before first use so the
                # scheduler interleaves them with the first block's compute
                for kb in blocks:
                    p = pair_idx[(tci, kb)]
                    if not m_loaded[p]:
                        nc.sync.dma_start(m_t[:, p, :], m_d[:, p, :])
                        m_loaded[p] = True
                acc = ppool.tile([128, 512], F32, name="acc")
                for j, kb in enumerate(blocks):
                    nc.tensor.matmul(
                        acc[:],
                        sig[:, kb, :],
                        m_t[:, pair_idx[(tci, kb)], :],
                        start=(j == 0), stop=(j == len(blocks) - 1),
                    )
                j = tci % ogroup
                if j == 0:
                    o_t = opool.tile([128, ogroup, 512], F32, name="o_t")
                if ncopy % 3 < 2:
                    nc.vector.tensor_copy(o_t[:, j, :], acc[:])
                else:
                    nc.scalar.copy(o_t[:, j, :], acc[:])
                ncopy += 1
                if j == ogroup - 1:
                    # batched store (OGROUP t-chunks -> one DMA, 8 KiB rows)
                    # on the Activation HWDGE queue so it doesn't
                    # head-of-line-block the input loads on the SP queue
                    t0 = (tci - j) * 128
                    nc.scalar.dma_start(
                        o_d[b, nb * 128:(nb + 1) * 128, t0:t0 + ogroup * 128, :],
                        o_t[:],
                    )


def build_module(reps=1, ogroup=None):
    """Build + compile the per-core Bass module.  reps>1 wraps the body in a
    hardware loop (used by test.py for wall-clock differencing timing)."""
    if ogroup is None:
        ogroup = OGROUP
    nc = bacc.Bacc("TRN2", target_bir_lowering=False, debug=False)
    x_d = nc.dram_tensor("x", [B_PER_CORE, N // 128, 128, TCHUNKS * 128],
                         SIG_DTYPE, kind="ExternalInput")
    m_d = nc.dram_tensor("m", [128, NPAIRS, 512], M_DTYPE, kind="ExternalInput")
    o_d = nc.dram_tensor("out", [B_PER_CORE, N, T, C], F32, kind="ExternalOutput")

    with tile.TileContext(nc) as tc_ctx, ExitStack() as ctx:
        pools = (
            ctx.enter_context(tc_ctx.tile_pool(name="mpool", bufs=1)),
            ctx.enter_context(tc_ctx.tile_pool(name="spool", bufs=3)),
            ctx.enter_context(tc_ctx.tile_pool(name="opool",
                                             bufs=max(3, 24 // ogroup))),
            ctx.enter_context(tc_ctx.tile_pool(name="ppool", bufs=8, space="PSUM")),
        )
        if reps == 1:
            _emit_body(tc_ctx, nc, x_d, m_d, o_d, pools, ogroup)
        else:
            with tc_ctx.For_i(0, reps, 1,
                              hint_engines=(mybir.EngineType.PE,
                                            mybir.EngineType.SP)):
                _emit_body(tc_ctx, nc, x_d, m_d, o_d, pools, ogroup)

    nc.compile()
    return nc


_NC_CACHE = {}


def _get_module(reps=1, ogroup=None):
    key = (reps, ogroup)
    if key not in _NC_CACHE:
        _NC_CACHE[key] = build_module(reps, ogroup)
    return _NC_CACHE[key]


# ------------------------------------------------------------------ entrypoint
def run(x, reps=1, ogroup=None):
    """x: [16, 2048, 512, 1] float32 -> [16, 512, 2048, 4] float32."""
    nc = _get_module(reps, ogroup)
    x3 = np.asarray(x)[:, :, :, 0]
    if SIG_DTYPE == BF16:
        import ml_dtypes
        x3 = x3.astype(ml_dtypes.bfloat16)
    else:
        x3 = x3.astype(np.float32)
    # pre-tile to the SBUF layout: [b, nb, tp, (kt n)]
    xt = np.ascontiguousarray(
        x3.reshape(B, TCHUNKS, 128, N // 128, 128).transpose(0, 3, 2, 1, 4)
        .reshape(B, N // 128, 128, TCHUNKS * 128))
    in_maps = [
        {"x": xt[c * B_PER_CORE:(c + 1) * B_PER_CORE], "m": _M_PACKED}
        for c in range(NCORES)
    ]
    res = run_bass_kernel_spmd(nc, in_maps, core_ids=list(range(NCORES)))
    out = np.concatenate([res.results[c]["out"] for c in range(NCORES)], axis=0)
    return out


def kernel(x):
    return run(x)



# revision 25
# speedup vs baseline: 1.5018x; 1.5018x over previous
"""Trainium2 Bass kernel for nn_DiscreteWaveletTransform (3-level db4 DWT,
symmetric padding, + linear resize of each coefficient band back to T).

Approach: the whole per-signal pipeline (3 DWT levels + 4 resizes) is one
fixed linear operator out[t, c] = sum_k sig[k] * M[k, 4t+c].  M (2048 x 8192)
is banded: the 512 output columns of one 128-wide t-chunk depend on <= 194
consecutive k rows, and the band start advances by exactly 128 k per t-chunk.
Blocking k on a grid shifted by SHIFT (so every t-chunk's band falls in two
consecutive 128-row blocks) gives exactly 2 PE matmuls per t-chunk:
[K=128, M=128 signals, N=512 cols] in bf16, accumulated in PSUM.  The matmul
orientation puts signals on PSUM partitions and (t, c)-interleaved columns on
the free axis, so each PSUM tile is a contiguous [128 signals, 128 t x 4 c]
chunk of the output layout — no transposes, contiguous output DMAs.

The output is stored to DRAM in bf16 (halves the dominant HBM stream) and
upcast to float32 on the host during the gather.

Sharding: data-parallel over B (16 -> 2 per core); each core handles
2 x 512 = 1024 signals = 8 blocks of 128.
"""

from contextlib import ExitStack

import numpy as np

import concourse.bacc as bacc
import concourse.bass as bass
import concourse.tile as tile
from concourse import mybir
from concourse.bass_utils import run_bass_kernel_spmd

# ---------------------------------------------------------------- problem dims
import os  # noqa: E402

B, T, N = 16, 2048, 512
LEVELS = 3
C = LEVELS + 1
F = 8
NCORES = 8
B_PER_CORE = B // NCORES          # 2
TCHUNKS = T // 128                # 16
SBLOCKS = B_PER_CORE * (N // 128)  # 8 signal blocks of 128 per core
OGROUP = int(os.environ.get("K_OGROUP", "8"))  # t-chunks per output DMA
OUT_RING = os.environ.get("K_OUT_RING", "alt")   # act | alt  (output DMA queue)
MPREFETCH = os.environ.get("K_MPREFETCH", "jit")  # jit | top  (M load placement)
COPY_SPLIT = int(os.environ.get("K_COPY_SPLIT", "2"))  # n-1 of n copies on DVE
OBUFS = int(os.environ.get("K_OBUFS", "0"))      # opool bufs override (0=auto)
MCOMPRESS = int(os.environ.get("K_MCOMPRESS", "0"))  # DMA/matmul only nonzero M rows
SIGSPLIT = int(os.environ.get("K_SIGSPLIT", "0"))    # split first sig load
PROBE = os.environ.get("K_PROBE", "full")  # full | dma | pe  (timing probes)
STAGGER = int(os.environ.get("K_STAGGER", "1"))  # staggered sem reset in For_i
MGROUP = int(os.environ.get("K_MGROUP", "2"))    # M pairs per load DMA (2|8|32)
SIGPAIR = int(os.environ.get("K_SIGPAIR", "0"))  # 2 signal blocks per sig DMA
SBUFS = int(os.environ.get("K_SBUFS", "3"))      # spool bufs (sig prefetch depth)
MBUFS = int(os.environ.get("K_MBUFS", "1"))      # mpool bufs (cross-rep M decouple)
SHIFT = 60                         # k-grid shift aligning blocks to the band
NBLK = (T + SHIFT + 127) // 128    # 17 shifted k-blocks cover [-SHIFT, 2116)

DEC_LO = np.array([-0.010597401784997278, 0.032883011666982945, 0.030841381835986965,
                   -0.18703481171888114, -0.02798376941698385, 0.6308807679295904,
                   0.7148465705525415, 0.23037781330885523])
DEC_HI = np.array([-0.23037781330885523, 0.7148465705525415, -0.6308807679295904,
                   -0.02798376941698385, 0.18703481171888114, 0.030841381835986965,
                   -0.032883011666982945, -0.010597401784997278])


# ------------------------------------------------------- operator construction
def _dwt_step(sig, lo, hi):
    S = sig.shape[1]
    ext = np.pad(sig, ((0, 0), (F - 1, F - 1)), mode='symmetric')[:, 1:]
    L = (S + F - 1) // 2
    lo_r, hi_r = lo[::-1], hi[::-1]
    cA = sum(ext[:, k:k + 2 * L:2] * lo_r[k] for k in range(F))
    cD = sum(ext[:, k:k + 2 * L:2] * hi_r[k] for k in range(F))
    return cA, cD


def _resize(c, t):
    S = c.shape[-1]
    if S == t:
        return c
    if S > t:
        return c[..., :t]
    pos = (np.arange(t, dtype=c.dtype) + 0.5) * (S / t) - 0.5
    pos = np.clip(pos, 0.0, S - 1)
    lo = np.floor(pos).astype(np.int64)
    hi = np.minimum(lo + 1, S - 1)
    w = pos - lo.astype(c.dtype)
    return c[..., lo] * (1.0 - w) + c[..., hi] * w


def _build_operator():
    """M [T, T, C] float64: out[s, t, c] = sum_k sig[s, k] M[k, t, c]."""
    a = np.eye(T)
    details = []
    for _ in range(LEVELS):
        a, d = _dwt_step(a, DEC_LO, DEC_HI)
        details.append(d)
    coeffs = [a] + details[::-1]
    return np.stack([_resize(cf, T) for cf in coeffs], axis=-1)


def _plan():
    """Shifted-banded matmul schedule.  Shifted k-block j covers rows
    [128j - SHIFT, 128j - SHIFT + 128); every t-chunk's band fits in blocks
    (j0, j0+1).  Returns (j0 per t-chunk, packed M [128, 2*TCHUNKS, 512])."""
    M = _build_operator()                      # [k, t, c]
    Mi = M.reshape(T, T * C)                   # col = 4 t + c
    j0s = []
    ranges = []                                # per-pair nonzero row range
    packed = np.zeros((128, 2 * TCHUNKS, 512), dtype=np.float32)
    for tc in range(TCHUNKS):
        cols = Mi[:, tc * 512:(tc + 1) * 512]
        rows = np.nonzero(np.any(cols != 0, axis=1))[0]
        j0 = (rows.min() + SHIFT) // 128
        assert rows.max() < 128 * (j0 + 2) - SHIFT, (tc, rows.min(), rows.max())
        j0s.append(j0)
        for i, j in enumerate((j0, j0 + 1)):
            k_lo = max(0, 128 * j - SHIFT)
            k_hi = min(T, 128 * (j + 1) - SHIFT)
            p_lo = k_lo - (128 * j - SHIFT)
            packed[p_lo:p_lo + (k_hi - k_lo), 2 * tc + i, :] = \
                Mi[k_lo:k_hi, tc * 512:(tc + 1) * 512]
            # nonzero band restricted to this block; base partition must be
            # a multiple of 32 for the PE (rows below stay zero-padded)
            b_lo = (max(k_lo, rows.min()) - (128 * j - SHIFT)) & ~31
            b_hi = min(k_hi - 1, rows.max()) - (128 * j - SHIFT) + 1
            ranges.append((b_lo, b_hi))
    return j0s, packed, ranges


_J0S, _M_PACKED, _PRANGES = _plan()
NPAIRS = 2 * TCHUNKS

F32 = mybir.dt.float32
BF16 = mybir.dt.bfloat16

import ml_dtypes  # noqa: E402

_M_PACKED = _M_PACKED.astype(ml_dtypes.bfloat16)


# ------------------------------------------------------------- device program
def _emit_body(tc_ctx, nc, x_d, m_d, o_d, pools, ogroup=None):
    mpool, spool, opool, ppool = pools

    m_t = mpool.tile([128, NPAIRS, 512], BF16, name="m_t")
    n_mgroups = (NPAIRS + MGROUP - 1) // MGROUP
    m_loaded = [False] * n_mgroups

    def load_m(eng, tci):
        g = (2 * tci) // MGROUP
        if not m_loaded[g]:
            eng.dma_start(m_t[:, g * MGROUP:(g + 1) * MGROUP, :],
                          m_d[:, g * MGROUP:(g + 1) * MGROUP, :])
            m_loaded[g] = True

    if MPREFETCH == "top":
        # all M loads up-front on the ACT ring (idle until first outputs)
        for tci in range(TCHUNKS):
            load_m(nc.scalar, tci)
    ncopy = 0
    nout = 0
    sigw = 2 if SIGPAIR else 1  # signal blocks per sig tile/DMA
    for b in range(B_PER_CORE):
        for nbp in range(N // (128 * sigw)):
            sig = spool.tile([128, sigw, NBLK, 128], BF16, name="sig")
            nc.sync.dma_start(sig[:], x_d[b, nbp])
            for blk in range(sigw):
                nb = nbp * sigw + blk
                for tci in range(TCHUNKS):
                    # just-in-time M loads, emitted right before first use so
                    # the scheduler interleaves them with the first block's
                    # compute; resident afterwards
                    load_m(nc.sync, tci)
                    if PROBE == "dma":
                        # stores stream straight from m_t: loads + stores only
                        j = tci % ogroup
                        if j == ogroup - 1:
                            t0 = (tci - j) * 128
                            oeng = nc.scalar if nout % 2 == 0 else nc.sync
                            oeng.dma_start(
                                o_d[b, nb * 128:(nb + 1) * 128,
                                    t0:t0 + ogroup * 128, :],
                                m_t[:, 0:ogroup, :])
                            nout += 1
                        continue
                    acc = ppool.tile([128, 512], F32, name="acc")
                    j0 = _J0S[tci]
                    nc.tensor.matmul(acc[:], sig[:, blk, j0, :],
                                     m_t[:, 2 * tci, :],
                                     start=True, stop=False)
                    nc.tensor.matmul(acc[:], sig[:, blk, j0 + 1, :],
                                     m_t[:, 2 * tci + 1, :],
                                     start=False, stop=True)
                    if PROBE == "pe":
                        continue
                    j = tci % ogroup
                    if j == 0:
                        o_t = opool.tile([128, ogroup, 512], BF16, name="o_t")
                    if ncopy % COPY_SPLIT != COPY_SPLIT - 1:
                        nc.vector.tensor_copy(o_t[:, j, :], acc[:])
                    else:
                        nc.scalar.copy(o_t[:, j, :], acc[:])
                    ncopy += 1
                    if j == ogroup - 1:
                        # batched store (OGROUP t-chunks -> one DMA), off the
                        # SP ring (or alternating rings) so stores don't
                        # head-of-line block the input loads
                        t0 = (tci - j) * 128
                        oeng = nc.scalar
                        if OUT_RING == "alt" and nout % 2 == 1:
                            oeng = nc.sync
                        elif OUT_RING == "rot3":
                            oeng = (nc.scalar, nc.sync, nc.gpsimd)[nout % 3]
                        oeng.dma_start(
                            o_d[b, nb * 128:(nb + 1) * 128,
                                t0:t0 + ogroup * 128, :],
                            o_t[:],
                        )
                        nout += 1


def build_module(reps=1, ogroup=None):
    """Build + compile the per-core Bass module.  reps>1 wraps the body in a
    hardware loop (used by test.py for wall-clock differencing timing)."""
    if ogroup is None:
        ogroup = OGROUP
    nc = bacc.Bacc("TRN2", target_bir_lowering=False, debug=False)
    sigw = 2 if SIGPAIR else 1
    x_d = nc.dram_tensor("x", [B_PER_CORE, N // (128 * sigw), 128,
                               sigw * NBLK * 128],
                         BF16, kind="ExternalInput")
    m_d = nc.dram_tensor("m", [128, NPAIRS, 512], BF16, kind="ExternalInput")
    o_d = nc.dram_tensor("out", [B_PER_CORE, N, T, C], BF16,
                         kind="ExternalOutput")

    with tile.TileContext(nc) as tc_ctx, ExitStack() as ctx:
        pools = (
            ctx.enter_context(tc_ctx.tile_pool(name="mpool", bufs=MBUFS)),
            ctx.enter_context(tc_ctx.tile_pool(name="spool", bufs=SBUFS)),
            ctx.enter_context(tc_ctx.tile_pool(name="opool",
                                             bufs=OBUFS or max(3, 24 // ogroup))),
            ctx.enter_context(tc_ctx.tile_pool(name="ppool", bufs=8, space="PSUM")),
        )
        if reps == 1:
            _emit_body(tc_ctx, nc, x_d, m_d, o_d, pools, ogroup)
        else:
            with tc_ctx.For_i(0, reps, 1,
                              hint_engines=(mybir.EngineType.PE,
                                            mybir.EngineType.SP),
                              staggered_reset=bool(STAGGER)):
                _emit_body(tc_ctx, nc, x_d, m_d, o_d, pools, ogroup)

    nc.compile()
    return nc


_NC_CACHE = {}


def _get_module(reps=1, ogroup=None):
    key = (reps, ogroup)
    if key not in _NC_CACHE:
        _NC_CACHE[key] = build_module(reps, ogroup)
    return _NC_CACHE[key]


# ------------------------------------------------------------------ entrypoint
def run(x, reps=1, ogroup=None):
    """x: [16, 2048, 512, 1] float32 -> [16, 512, 2048, 4] float32."""
    nc = _get_module(reps, ogroup)
    x3 = np.asarray(x)[:, :, :, 0].astype(ml_dtypes.bfloat16)  # [B, T, N]
    # shifted k grid: block j, partition p holds k = 128j - SHIFT + p
    xp = np.zeros((B, NBLK * 128, N), dtype=ml_dtypes.bfloat16)
    xp[:, SHIFT:SHIFT + T, :] = x3
    # pre-tile to the SBUF layout: [b, nb_group, p, (blk j n)]
    sigw = 2 if SIGPAIR else 1
    xt = np.ascontiguousarray(
        xp.reshape(B, NBLK, 128, N // (128 * sigw), sigw, 128)
        .transpose(0, 3, 2, 4, 1, 5)
        .reshape(B, N // (128 * sigw), 128, sigw * NBLK * 128))
    in_maps = [
        {"x": xt[c * B_PER_CORE:(c + 1) * B_PER_CORE], "m": _M_PACKED}
        for c in range(NCORES)
    ]
    res = run_bass_kernel_spmd(nc, in_maps, core_ids=list(range(NCORES)))
    out = np.concatenate([res.results[c]["out"] for c in range(NCORES)], axis=0)
    return out.astype(np.float32)


def kernel(x):
    return run(x)
